# revision 1
# baseline (speedup 1.0000x reference)
"""Mamba SSM block on 8 TRN2 NeuronCores (Bass/Tile, SPMD).

Sharding: d_inner (2048 -> 256/core) across in_proj, conv, dt/B/C projections
and the selective scan (all per-core local). Two small collectives per
512-token chunk:
  - AllReduce of x_dbl projection partials [96, 512] fp32
  - AllGather of the gated scan output yg [256, 512] bf16 (pure copy), after
    which each core computes only its 128 rows of the (host-folded)
    W_c = W_out @ W_out_ssm output projection.

Scan: h[t] = exp(A dt[t]) h[t-1] + (dt[t] x[t]) B[t] via 16 independent
tensor_tensor_scan ops (one per state index) along tokens, chained across
chunks through per-partition `initial` APs. Matmul operands and scan tensors
are bf16 (rel_err ~4e-3 validated vs fp32 reference); conv, activations, dt
and the AllReduce stay fp32.
"""
import numpy as np
import ml_dtypes

import concourse.bass as bass
import concourse.tile as tile
from concourse import bacc, mybir
from concourse.bass_utils import run_bass_kernel_spmd

BFnp = ml_dtypes.bfloat16
F32 = mybir.dt.float32
BF16 = mybir.dt.bfloat16
AF = mybir.ActivationFunctionType
OP = mybir.AluOpType

NC = 8
B, L, DM = 2, 2048, 1024
DI, S, R, KC = 2048, 16, 64, 4
DIL = DI // NC            # 256 d_inner per core
NT = B * L                # 4096 tokens (batch-major)
TC = 512                  # tokens per chunk
NCH = NT // TC            # 8 chunks
EL = DM // NC             # 128 output rows per core
NI = DIL // 128           # 2 partition tiles of local d_inner

_NC_CACHE = {}


def build():
    if "nc" in _NC_CACHE:
        return _NC_CACHE["nc"]
    nc = bacc.Bacc("TRN2", target_bir_lowering=False, debug=False, num_devices=NC)

    # ---- per-core DRAM inputs (host pre-sharded / transposed / casted) ----
    x_t = nc.dram_tensor("x_t", [DM, NT], BF16, kind="ExternalInput")         # replicated
    w_in_x = nc.dram_tensor("w_in_x", [DM, DIL], BF16, kind="ExternalInput")  # W_in[dk,:].T
    w_in_z = nc.dram_tensor("w_in_z", [DM, DIL], BF16, kind="ExternalInput")
    conv_w = nc.dram_tensor("conv_w", [DIL, KC], F32, kind="ExternalInput")
    conv_b = nc.dram_tensor("conv_b", [DIL, 1], F32, kind="ExternalInput")
    w_xp = nc.dram_tensor("w_xp", [DIL, R + 2 * S], BF16, kind="ExternalInput")  # W_xp[:,dk].T
    w_dt = nc.dram_tensor("w_dt", [R, DIL], BF16, kind="ExternalInput")          # W_dt[dk,:].T
    b_dt = nc.dram_tensor("b_dt", [DIL, 1], F32, kind="ExternalInput")
    a_mat = nc.dram_tensor("a_mat", [DIL, S], F32, kind="ExternalInput")        # -exp(A_log[dk])
    d_vec = nc.dram_tensor("d_vec", [DIL, 1], F32, kind="ExternalInput")
    w_c = nc.dram_tensor("w_c", [DI, EL], BF16, kind="ExternalInput")           # W_c[ek,:].T
    b_o = nc.dram_tensor("b_o", [EL, 1], F32, kind="ExternalInput")
    sel_in = nc.dram_tensor("sel_in", [S, S * 128], BF16, kind="ExternalInput")
    out = nc.dram_tensor("out", [NCH, EL, TC], F32, kind="ExternalOutput")

    with tile.TileContext(nc) as tc:
        with (
            tc.tile_pool(name="wpool", bufs=1) as wp,     # persistent weights
            tc.tile_pool(name="xpool", bufs=2) as xp,
            tc.tile_pool(name="xckp", bufs=1) as xkp,     # streamed x / yg gather
            tc.tile_pool(name="work", bufs=1) as wk,      # DVE-only transients
            tc.tile_pool(name="worka", bufs=2) as wka,    # ACT/DMA-written tiles
            tc.tile_pool(name="keep", bufs=6) as kp,      # xs/g (live across chunk)
            tc.tile_pool(name="scan", bufs=1) as sc,      # big bf16 scan tiles
            tc.tile_pool(name="scana", bufs=2) as sca,    # a (ACT-written, dbl buf)
            tc.tile_pool(name="bcast", bufs=1) as bcp,    # B/C broadcast tiles
            tc.tile_pool(name="state", bufs=1) as st,     # persistent hprev/xtail
            tc.tile_pool(name="psA", bufs=2, space="PSUM") as psA,
            tc.tile_pool(name="psB", bufs=1, space="PSUM") as psB,
            tc.tile_pool(name="dram", bufs=4, space="DRAM") as dr,
        ):
            # ---------- load weights ----------
            winx = wp.tile([128, 8 * NI * 128], BF16, tag="winx")
            winz = wp.tile([128, 8 * NI * 128], BF16, tag="winz")
            for kt in range(8):
                for i in range(NI):
                    nc.sync.dma_start(
                        winx[:, (kt * NI + i) * 128:(kt * NI + i + 1) * 128],
                        w_in_x[kt * 128:(kt + 1) * 128, i * 128:(i + 1) * 128])
                    nc.sync.dma_start(
                        winz[:, (kt * NI + i) * 128:(kt * NI + i + 1) * 128],
                        w_in_z[kt * 128:(kt + 1) * 128, i * 128:(i + 1) * 128])
            wxp = wp.tile([128, NI * (R + 2 * S)], BF16, tag="wxp")
            for i in range(NI):
                nc.sync.dma_start(
                    wxp[:, i * (R + 2 * S):(i + 1) * (R + 2 * S)],
                    w_xp[i * 128:(i + 1) * 128, :])
            wdt = wp.tile([R, NI * 128], BF16, tag="wdt")
            nc.sync.dma_start(wdt[:], w_dt[:, :])
            wc = wp.tile([128, 16 * EL], BF16, tag="wc")
            for kt in range(16):
                nc.sync.dma_start(
                    wc[:, kt * EL:(kt + 1) * EL],
                    w_c[kt * 128:(kt + 1) * 128, :])
            cw = wp.tile([128, NI * KC], F32, tag="cw")
            cb = wp.tile([128, NI], F32, tag="cb")
            bdt = wp.tile([128, NI], F32, tag="bdt")
            dv = wp.tile([128, NI], F32, tag="dv")
            am = wp.tile([128, NI * S], F32, tag="am")
            for i in range(NI):
                sl = slice(i * 128, (i + 1) * 128)
                nc.sync.dma_start(cw[:, i * KC:(i + 1) * KC], conv_w[sl, :])
                nc.sync.dma_start(cb[:, i:i + 1], conv_b[sl, :])
                nc.sync.dma_start(bdt[:, i:i + 1], b_dt[sl, :])
                nc.sync.dma_start(dv[:, i:i + 1], d_vec[sl, :])
                nc.sync.dma_start(am[:, i * S:(i + 1) * S], a_mat[sl, :])
            bo = wp.tile([EL, 1], F32, tag="bo")
            nc.sync.dma_start(bo[:], b_o[:, :])
            sel = wp.tile([S, S * 128], BF16, tag="sel")
            nc.sync.dma_start(sel[:], sel_in[:, :])

            hprev = st.tile([128, NI * S], BF16, tag="hprev")
            xtail = st.tile([128, NI * 3], F32, tag="xtail")

            ctx = {}

            def front(c):
                """x stream, in_proj, conv, silu, z-gate, x_dbl partial, AR issue."""
                t0 = c * TC
                reset = (c % (NCH // B) == 0)

                xck = xkp.tile([128, 8 * TC], BF16, tag="xck")
                for kt in range(8):
                    nc.sync.dma_start(
                        xck[:, kt * TC:(kt + 1) * TC],
                        x_t[kt * 128:(kt + 1) * 128, t0:t0 + TC])

                xs_i, g_i = [], []
                for i in range(NI):
                    ps_x = psA.tile([128, TC], F32, tag="psx")
                    ps_z = psA.tile([128, TC], F32, tag="psz")
                    for kt in range(8):
                        wsl = slice((kt * NI + i) * 128, (kt * NI + i + 1) * 128)
                        nc.tensor.matmul(ps_x[:], winx[:, wsl], xck[:, kt * TC:(kt + 1) * TC],
                                         start=(kt == 0), stop=(kt == 7))
                    for kt in range(8):
                        wsl = slice((kt * NI + i) * 128, (kt * NI + i + 1) * 128)
                        nc.tensor.matmul(ps_z[:], winz[:, wsl], xck[:, kt * TC:(kt + 1) * TC],
                                         start=(kt == 0), stop=(kt == 7))

                    # causal depthwise conv (taps read PSUM)
                    head = wk.tile([128, 7], F32, tag="head")
                    if reset:
                        nc.gpsimd.memset(head[:, 0:3], 0.0)
                    else:
                        nc.vector.tensor_copy(head[:, 0:3], xtail[:, i * 3:i * 3 + 3])
                    nc.vector.tensor_copy(head[:, 3:7], ps_x[:, 0:4])
                    nc.vector.tensor_copy(xtail[:, i * 3:i * 3 + 3], ps_x[:, TC - 3:TC])

                    acc0 = wk.tile([128, TC], F32, tag="accA")
                    nc.vector.tensor_scalar_mul(acc0[:, 3:], ps_x[:, 0:TC - 3], cw[:, i * KC:i * KC + 1])
                    acc1 = wk.tile([128, TC], F32, tag="accB")
                    nc.vector.scalar_tensor_tensor(
                        out=acc1[:, 3:], in0=ps_x[:, 1:TC - 2], scalar=cw[:, i * KC + 1:i * KC + 2],
                        in1=acc0[:, 3:], op0=OP.mult, op1=OP.add)
                    acc2 = wk.tile([128, TC], F32, tag="accA")
                    nc.vector.scalar_tensor_tensor(
                        out=acc2[:, 3:], in0=ps_x[:, 2:TC - 1], scalar=cw[:, i * KC + 2:i * KC + 3],
                        in1=acc1[:, 3:], op0=OP.mult, op1=OP.add)
                    u = wk.tile([128, TC], F32, tag="accB")
                    nc.vector.scalar_tensor_tensor(
                        out=u[:, 3:], in0=ps_x[:, 3:TC], scalar=cw[:, i * KC + 3:i * KC + 4],
                        in1=acc2[:, 3:], op0=OP.mult, op1=OP.add)
                    nc.vector.tensor_scalar_mul(acc0[:, 0:3], head[:, 0:3], cw[:, i * KC:i * KC + 1])
                    nc.vector.scalar_tensor_tensor(
                        out=acc1[:, 0:3], in0=head[:, 1:4], scalar=cw[:, i * KC + 1:i * KC + 2],
                        in1=acc0[:, 0:3], op0=OP.mult, op1=OP.add)
                    nc.vector.scalar_tensor_tensor(
                        out=acc2[:, 0:3], in0=head[:, 2:5], scalar=cw[:, i * KC + 2:i * KC + 3],
                        in1=acc1[:, 0:3], op0=OP.mult, op1=OP.add)
                    nc.vector.scalar_tensor_tensor(
                        out=u[:, 0:3], in0=head[:, 3:6], scalar=cw[:, i * KC + 3:i * KC + 4],
                        in1=acc2[:, 0:3], op0=OP.mult, op1=OP.add)
                    sgu = wka.tile([128, TC], F32, tag="act1")
                    nc.scalar.activation(sgu[:], u[:], AF.Sigmoid, bias=cb[:, i:i + 1])
                    xs = kp.tile([128, TC], BF16, tag="xs")
                    nc.vector.scalar_tensor_tensor(
                        out=xs[:], in0=u[:], scalar=cb[:, i:i + 1], in1=sgu[:],
                        op0=OP.add, op1=OP.mult)
                    xs_i.append(xs)

                    # z gate: g = z * sigmoid(z)
                    sgz = wka.tile([128, TC], F32, tag="act3")
                    nc.scalar.activation(sgz[:], ps_z[:], AF.Sigmoid)
                    g = kp.tile([128, TC], BF16, tag="g")
                    nc.vector.scalar_tensor_tensor(
                        out=g[:], in0=sgz[:], scalar=1.0, in1=ps_z[:],
                        op0=OP.mult, op1=OP.mult)
                    g_i.append(g)

                # x_dbl partial + AllReduce
                ps_xd = psB.tile([R + 2 * S, TC], F32, tag="psxd")
                for i in range(NI):
                    nc.tensor.matmul(ps_xd[:], wxp[:, i * (R + 2 * S):(i + 1) * (R + 2 * S)],
                                     xs_i[i][:], start=(i == 0), stop=(i == NI - 1))
                xd_sb = wka.tile([R + 2 * S, TC], BF16, tag="xdsb")
                nc.scalar.copy(xd_sb[:], ps_xd[:])
                xd_part = dr.tile([R + 2 * S, TC], BF16, tag="xdp")
                nc.sync.dma_start(xd_part[:], xd_sb[:])
                xd_red = nc.dram_tensor(f"xd_red_{c}", [R + 2 * S, TC], BF16, addr_space="Shared")
                nc.gpsimd.collective_compute(
                    "AllReduce", OP.add, replica_groups=[list(range(NC))],
                    ins=[xd_part[:]], outs=[xd_red.ap()])
                ctx[c] = dict(xs_i=xs_i, g_i=g_i, xd_red=xd_red)

            def mid(c):
                """dtr/B/C loads, dt, a-exps, bb, scan, hc, tree, gate, yg, AG issue."""
                reset = (c % (NCH // B) == 0)
                xs_i = ctx[c]["xs_i"]
                g_i = ctx[c]["g_i"]
                xd_red = ctx[c]["xd_red"]

                dtr = wka.tile([R, TC], BF16, tag="dtr")
                nc.sync.dma_start(dtr[:], xd_red.ap()[0:R, :])
                brows = wka.tile([S, TC], BF16, tag="brows")
                nc.sync.dma_start(brows[:], xd_red.ap()[R:R + S, :])
                crows = wka.tile([S, TC], BF16, tag="crows")
                nc.sync.dma_start(crows[:], xd_red.ap()[R + S:R + 2 * S, :])
                b_bc = bcp.tile([128, S * TC], BF16, tag="bbc")
                c_bc = bcp.tile([128, S * TC], BF16, tag="cbc")
                for s in range(S):
                    ps_bc = psB.tile([128, TC], F32, tag="psbc")
                    nc.tensor.matmul(ps_bc[:], sel[:, s * 128:(s + 1) * 128],
                                     brows[:], start=True, stop=True)
                    nc.scalar.copy(b_bc[:, s * TC:(s + 1) * TC], ps_bc[:])
                    ps_cc = psB.tile([128, TC], F32, tag="psbc")
                    nc.tensor.matmul(ps_cc[:], sel[:, s * 128:(s + 1) * 128],
                                     crows[:], start=True, stop=True)
                    nc.scalar.copy(c_bc[:, s * TC:(s + 1) * TC], ps_cc[:])

                yg_part = dr.tile([DIL, TC], BF16, tag="ygp")
                for i in range(NI):
                    ps_dt = psB.tile([128, TC], F32, tag="psdt")
                    nc.tensor.matmul(ps_dt[:], wdt[:, i * 128:(i + 1) * 128], dtr[:],
                                     start=True, stop=True)
                    edt = wka.tile([128, TC], F32, tag="act2")
                    nc.scalar.activation(edt[:], ps_dt[:], AF.Exp, bias=bdt[:, i:i + 1])
                    dt = wka.tile([128, TC], F32, tag="dtt")
                    nc.scalar.activation(dt[:], edt[:], AF.Ln, bias=1.0)

                    dtx = wk.tile([128, TC], BF16, tag="dtx")
                    nc.vector.tensor_tensor(out=dtx[:], in0=dt[:], in1=xs_i[i][:], op=OP.mult)

                    a_t = sca.tile([128, S * TC], BF16, tag="a_t")
                    for s in range(S):
                        nc.scalar.activation(
                            a_t[:, s * TC:(s + 1) * TC], dt[:], AF.Exp,
                            scale=am[:, i * S + s:i * S + s + 1])

                    bb_t = sc.tile([128, S * TC], BF16, tag="bb_t")
                    nc.vector.tensor_tensor(
                        out=bb_t[:].rearrange("p (s t) -> p s t", s=S),
                        in0=dtx[:].unsqueeze(1).broadcast_to([128, S, TC]),
                        in1=b_bc[:].rearrange("p (s t) -> p s t", s=S), op=OP.mult)

                    h_t = sc.tile([128, S * TC], BF16, tag="h_t")
                    if reset:
                        nc.gpsimd.memset(hprev[:, i * S:(i + 1) * S], 0.0)
                    for s in range(S):
                        nc.vector.tensor_tensor_scan(
                            h_t[:, s * TC:(s + 1) * TC],
                            a_t[:, s * TC:(s + 1) * TC],
                            bb_t[:, s * TC:(s + 1) * TC],
                            hprev[:, i * S + s:i * S + s + 1],
                            op0=OP.mult, op1=OP.add)
                    nc.sync.dma_start(
                        hprev[:, i * S:(i + 1) * S],
                        h_t[:].rearrange("p (s t) -> p s t", s=S)[:, :, TC - 1])

                    hc_t = sc.tile([128, S * TC], BF16, tag="hc_t")
                    nc.vector.tensor_tensor(out=hc_t[:], in0=h_t[:], in1=c_bc[:], op=OP.mult)
                    r1 = sc.tile([128, S * TC // 2], BF16, tag="bb_t")
                    nc.gpsimd.tensor_tensor(out=r1[:], in0=hc_t[:, :S * TC // 2],
                                            in1=hc_t[:, S * TC // 2:], op=OP.add)
                    r2 = sc.tile([128, S * TC // 4], BF16, tag="h_t")
                    nc.gpsimd.tensor_tensor(out=r2[:], in0=r1[:, :S * TC // 4],
                                            in1=r1[:, S * TC // 4:], op=OP.add)
                    r3 = sc.tile([128, S * TC // 8], BF16, tag="bb_t")
                    nc.vector.tensor_tensor(out=r3[:], in0=r2[:, :S * TC // 8],
                                            in1=r2[:, S * TC // 8:], op=OP.add)
                    y = wk.tile([128, TC], F32, tag="y")
                    nc.vector.tensor_tensor(out=y[:], in0=r3[:, :TC], in1=r3[:, TC:], op=OP.add)

                    yD = wk.tile([128, TC], F32, tag="yD")
                    nc.vector.scalar_tensor_tensor(
                        out=yD[:], in0=xs_i[i][:], scalar=dv[:, i:i + 1], in1=y[:],
                        op0=OP.mult, op1=OP.add)
                    yg = wk.tile([128, TC], BF16, tag="yg")
                    nc.vector.tensor_tensor(out=yg[:], in0=yD[:], in1=g_i[i][:], op=OP.mult)
                    nc.sync.dma_start(yg_part[i * 128:(i + 1) * 128, :], yg[:])

                yg_full = nc.dram_tensor(f"yg_full_{c}", [DI, TC], BF16, addr_space="Shared")
                nc.gpsimd.collective_compute(
                    "AllGather", OP.bypass, replica_groups=[list(range(NC))],
                    ins=[yg_part[:]], outs=[yg_full.ap()])
                ctx[c]["yg_full"] = yg_full

            def tail(c):
                """gather yg_full, out projection, bias, store."""
                yg_full = ctx[c]["yg_full"]
                ps_o = psB.tile([EL, TC], F32, tag="pso")
                for half in range(2):
                    ygs = xp.tile([128, 8 * TC], BF16, tag="ygs")
                    for j in range(8):
                        kt = half * 8 + j
                        nc.sync.dma_start(ygs[:, j * TC:(j + 1) * TC],
                                          yg_full.ap()[kt * 128:(kt + 1) * 128, :])
                    for j in range(8):
                        kt = half * 8 + j
                        nc.tensor.matmul(ps_o[:], wc[:, kt * EL:(kt + 1) * EL],
                                         ygs[:, j * TC:(j + 1) * TC],
                                         start=(kt == 0), stop=(kt == 15))
                o_sb = wk.tile([EL, TC], F32, tag="osb")
                nc.scalar.activation(o_sb[:], ps_o[:], AF.Identity, bias=bo[:])
                nc.sync.dma_start(out[c, :, :], o_sb[:])
                del ctx[c]

            for c in range(NCH + 3):
                if c < NCH:
                    front(c)
                if c >= 2 and c - 2 < NCH:
                    mid(c - 2)
                if c >= 3:
                    tail(c - 3)

    nc.compile()
    _NC_CACHE["nc"] = nc
    return nc


def _prep_inputs(inputs):
    x = np.ascontiguousarray(np.asarray(inputs["x"], np.float32))
    W_in = np.asarray(inputs["W_in"], np.float32)
    conv_w = np.asarray(inputs["conv_w"], np.float32)
    conv_b = np.asarray(inputs["conv_b"], np.float32)
    W_xp = np.asarray(inputs["W_xp"], np.float32)
    W_dt = np.asarray(inputs["W_dt"], np.float32)
    b_dt = np.asarray(inputs["b_dt"], np.float32)
    A_log = np.asarray(inputs["A_log"], np.float32)
    D = np.asarray(inputs["D"], np.float32)
    W_out_ssm = np.asarray(inputs["W_out_ssm"], np.float32)
    W_out = np.asarray(inputs["W_out"], np.float32)
    b_out = np.asarray(inputs["b_out"], np.float32)

    A = -np.exp(A_log)
    W_c = (W_out.astype(np.float64) @ W_out_ssm.astype(np.float64)).astype(np.float32)
    x_t = np.ascontiguousarray(x.reshape(NT, DM).T.astype(BFnp))  # [DM, NT] bf16
    sel_np = np.zeros((S, S * 128), BFnp)
    for s in range(S):
        sel_np[s, s * 128:(s + 1) * 128] = 1.0

    in_maps = []
    for k in range(NC):
        dsl = slice(k * DIL, (k + 1) * DIL)
        esl = slice(k * EL, (k + 1) * EL)
        in_maps.append({
            "x_t": x_t,
            "w_in_x": np.ascontiguousarray(W_in[dsl, :].T.astype(BFnp)),
            "w_in_z": np.ascontiguousarray(
                W_in[DI + k * DIL: DI + (k + 1) * DIL, :].T.astype(BFnp)),
            "conv_w": np.ascontiguousarray(conv_w[dsl, 0, :]),
            "conv_b": np.ascontiguousarray(conv_b[dsl][:, None]),
            "w_xp": np.ascontiguousarray(W_xp[:, dsl].T.astype(BFnp)),
            "w_dt": np.ascontiguousarray(W_dt[dsl, :].T.astype(BFnp)),
            "b_dt": np.ascontiguousarray(b_dt[dsl][:, None]),
            "a_mat": np.ascontiguousarray(A[dsl, :]),
            "d_vec": np.ascontiguousarray(D[dsl][:, None]),
            "w_c": np.ascontiguousarray(W_c[esl, :].T.astype(BFnp)),
            "b_o": np.ascontiguousarray(b_out[esl][:, None]),
            "sel_in": sel_np,
        })
    return in_maps


def _assemble(results):
    full = np.zeros((DM, NT), np.float32)
    for k in range(NC):
        o = results[k]["out"]  # [NCH, EL, TC]
        for c in range(NCH):
            full[k * EL:(k + 1) * EL, c * TC:(c + 1) * TC] = o[c]
    return np.ascontiguousarray(full.T).reshape(B, L, DM)


def kernel(**inputs):
    nc = build()
    in_maps = _prep_inputs(inputs)
    res = run_bass_kernel_spmd(nc, in_maps, core_ids=list(range(NC)))
    return _assemble(res.results)


def kernel_sim(**inputs):
    """Run through MultiCoreSim instead of HW (for debugging)."""
    from concourse.bass_interp import MultiCoreSim
    nc = build()
    in_maps = _prep_inputs(inputs)
    sim = MultiCoreSim(nc, num_cores=NC)
    for k in range(NC):
        for name, arr in in_maps[k].items():
            sim.cores[k].tensor(name)[:] = arr
    sim.simulate(check_with_hw=False)
    results = [{"out": sim.cores[k].tensor("out").copy()} for k in range(NC)]
    return _assemble(results)



# revision 11
# speedup vs baseline: 1.4558x; 1.4558x over previous
"""Mamba SSM block on 8 TRN2 NeuronCores (Bass/Tile, SPMD).

Sharding: 2-way batch DP x 4-way d_inner TP (512 channels/core, each core
processes only its batch's 2048 tokens in 2 chunks of 1024).

Per chunk: in_proj + causal conv + silu (sigmoid via exp + DVE fast
reciprocal, keeping a single ACT table: natural_log_exp), x_dbl partial +
AllReduce [96,1024] over the 4-core TP group. Scan phase: dt via
exp/ln softplus, per 4-state group: B/C row-broadcasts via DMA
(stride-0 partition replication), a_t = exp(A*dt) on ACT, bb on DVE 2x,
one merged tensor_tensor_scan per group (a=0 at segment heads folds the
per-state initial condition into the first bb element), h*C and a 2x
binary-tree reduce on DVE, with one state-group's scan on the Pool
engine. out_proj computes a full d_model partial per core (W_out @
W_out_ssm folded host-side) and a ReduceScatter replaces the baseline's
AllGather. All matmuls n=512 to keep 8 PSUM banks free-flowing.
"""
import numpy as np
import ml_dtypes

import concourse.bass as bass
import concourse.tile as tile
from concourse import bacc, mybir
from concourse.bass_utils import run_bass_kernel_spmd

BFnp = ml_dtypes.bfloat16
F32 = mybir.dt.float32
BF16 = mybir.dt.bfloat16
AF = mybir.ActivationFunctionType
OP = mybir.AluOpType

NC = 8
TPG = 4                   # tensor-parallel group size
B, L, DM = 2, 2048, 1024
DI, S, R, KC = 2048, 16, 64, 4
DIL = DI // TPG           # 512 d_inner per core
NI = DIL // 128           # 4 partition tiles
TC = 1024                 # tokens per chunk
NCH = L // TC             # 2 chunks (each core: only its batch)
SG = 4                    # states per scan group
NSG = S // SG             # 4 groups
EL = DM // TPG            # 256 output rows per core
NE = EL // 128
HTC = TC // 2             # matmul n (half chunk)
GROUPS = [[0, 1, 2, 3], [4, 5, 6, 7]]

POOL_SCAN_SG = set()      # Pool rejects TensorScalarPtr/scan at codegen

_NC_CACHE = {}


def build():
    if "nc" in _NC_CACHE:
        return _NC_CACHE["nc"]
    nc = bacc.Bacc("TRN2", target_bir_lowering=False, debug=False, num_devices=NC)

    # ---- per-core DRAM inputs (host pre-sharded / transposed / casted) ----
    x_t = nc.dram_tensor("x_t", [DM, L], BF16, kind="ExternalInput")       # own batch
    w_in_x = nc.dram_tensor("w_in_x", [DM, DIL], BF16, kind="ExternalInput")
    w_in_z = nc.dram_tensor("w_in_z", [DM, DIL], BF16, kind="ExternalInput")
    conv_w = nc.dram_tensor("conv_w", [DIL, KC], F32, kind="ExternalInput")
    conv_b = nc.dram_tensor("conv_b", [DIL, 1], F32, kind="ExternalInput")
    ncb_in = nc.dram_tensor("ncb_in", [DIL, 1], F32, kind="ExternalInput")  # -conv_b
    w_xp = nc.dram_tensor("w_xp", [DIL, R + 2 * S], BF16, kind="ExternalInput")
    w_dt = nc.dram_tensor("w_dt", [R, DIL], BF16, kind="ExternalInput")
    b_dt = nc.dram_tensor("b_dt", [DIL, 1], F32, kind="ExternalInput")
    a_mat = nc.dram_tensor("a_mat", [DIL, S], F32, kind="ExternalInput")   # -exp(A_log)
    d_vec = nc.dram_tensor("d_vec", [DIL, 1], F32, kind="ExternalInput")
    w_c = nc.dram_tensor("w_c", [DIL, DM], BF16, kind="ExternalInput")     # W_c[:,ch].T
    b_o = nc.dram_tensor("b_o", [EL, 1], F32, kind="ExternalInput")
    out = nc.dram_tensor("out", [EL, L], F32, kind="ExternalOutput")

    with tile.TileContext(nc) as tc:
        with (
            tc.tile_pool(name="wpool", bufs=1) as wp,     # persistent weights
            tc.tile_pool(name="xpool", bufs=2) as xp,     # x half-chunk stream
            tc.tile_pool(name="stage", bufs=1) as stg,    # xin / u / acc staging
            tc.tile_pool(name="work", bufs=2) as wk,      # silu chains etc.
            tc.tile_pool(name="work1", bufs=1) as wk1,    # single-buffered transients
            tc.tile_pool(name="keep", bufs=2 * NI) as kp,  # xs/g live across phases
            tc.tile_pool(name="midk", bufs=NI) as mk,     # dt/dtx/y_acc per chunk
            tc.tile_pool(name="scan", bufs=1) as sc,      # bb/h tiles
            tc.tile_pool(name="scana", bufs=2) as sca,    # a (ACT-written, dbl)
            tc.tile_pool(name="bcast", bufs=1) as bcp,    # B/C broadcast tiles
            tc.tile_pool(name="outp", bufs=2) as op_,     # yo tiles
            tc.tile_pool(name="ygp", bufs=NI) as ygp,     # yg (4 live per chunk)
            tc.tile_pool(name="state", bufs=1) as st,     # hprev / xtail
            tc.tile_pool(name="psin", bufs=2, space="PSUM") as psin,
            tc.tile_pool(name="psxd", bufs=2, space="PSUM") as psxd,
            tc.tile_pool(name="psdt", bufs=2, space="PSUM") as psdt,
            tc.tile_pool(name="pso", bufs=2, space="PSUM") as pso,
            tc.tile_pool(name="dram", bufs=2, space="DRAM") as dr,
        ):
            # ---------- load weights ----------
            winx = wp.tile([128, 8 * NI * 128], BF16, tag="winx")
            winz = wp.tile([128, 8 * NI * 128], BF16, tag="winz")
            for kt in range(8):
                for i in range(NI):
                    csl = slice((kt * NI + i) * 128, (kt * NI + i + 1) * 128)
                    nc.sync.dma_start(winx[:, csl],
                                      w_in_x[kt * 128:(kt + 1) * 128, i * 128:(i + 1) * 128])
                    nc.sync.dma_start(winz[:, csl],
                                      w_in_z[kt * 128:(kt + 1) * 128, i * 128:(i + 1) * 128])
            wxp = wp.tile([128, NI * (R + 2 * S)], BF16, tag="wxp")
            for i in range(NI):
                nc.sync.dma_start(wxp[:, i * (R + 2 * S):(i + 1) * (R + 2 * S)],
                                  w_xp[i * 128:(i + 1) * 128, :])
            wdt = wp.tile([R, DIL], BF16, tag="wdt")
            nc.sync.dma_start(wdt[:], w_dt[:, :])
            wc = wp.tile([128, NI * 8 * 128], BF16, tag="wc")
            for i in range(NI):
                for mt in range(8):
                    nc.sync.dma_start(
                        wc[:, (i * 8 + mt) * 128:(i * 8 + mt + 1) * 128],
                        w_c[i * 128:(i + 1) * 128, mt * 128:(mt + 1) * 128])
            cw = wp.tile([128, NI * KC], F32, tag="cw")
            cb = wp.tile([128, NI], F32, tag="cb")
            ncb = wp.tile([128, NI], F32, tag="ncb")
            bdt = wp.tile([128, NI], F32, tag="bdt")
            dv = wp.tile([128, NI], F32, tag="dv")
            am = wp.tile([128, NI * S], F32, tag="am")
            for i in range(NI):
                sl = slice(i * 128, (i + 1) * 128)
                nc.sync.dma_start(cw[:, i * KC:(i + 1) * KC], conv_w[sl, :])
                nc.sync.dma_start(cb[:, i:i + 1], conv_b[sl, :])
                nc.sync.dma_start(ncb[:, i:i + 1], ncb_in[sl, :])
                nc.sync.dma_start(bdt[:, i:i + 1], b_dt[sl, :])
                nc.sync.dma_start(dv[:, i:i + 1], d_vec[sl, :])
                nc.sync.dma_start(am[:, i * S:(i + 1) * S], a_mat[sl, :])
            bo = wp.tile([128, NE], F32, tag="bo")
            for mt in range(NE):
                nc.sync.dma_start(bo[:, mt:mt + 1], b_o[mt * 128:(mt + 1) * 128, :])

            hprev = st.tile([128, NI * S], BF16, tag="hprev")
            xtail = st.tile([128, NI * 3], F32, tag="xtail")

            ctx = {}

            def front(c):
                """x stream, in_proj, conv, silu, z-gate, x_dbl partial, AR."""
                t0 = c * TC
                xs_i, g_i = [], []
                ps_xd = [None, None]
                for i in range(NI):
                    # -- in_proj x branch, both halves -> xin staging (f32)
                    xin = stg.tile([128, TC + 3], F32, tag="xin")
                    if c == 0:
                        nc.gpsimd.memset(xin[:, 0:3], 0.0)
                    else:
                        nc.gpsimd.tensor_copy(xin[:, 0:3], xtail[:, i * 3:i * 3 + 3])
                    for h in range(2):
                        xck = xp.tile([128, 8 * HTC], BF16, tag="xck")
                        if i == 0:
                            for kt in range(8):
                                nc.sync.dma_start(
                                    xck[:, kt * HTC:(kt + 1) * HTC],
                                    x_t[kt * 128:(kt + 1) * 128,
                                        t0 + h * HTC:t0 + (h + 1) * HTC])
                            ctx[(c, "xck", h)] = xck
                        else:
                            xck = ctx[(c, "xck", h)]
                        ps_x = psin.tile([128, HTC], F32, tag="ps")
                        for kt in range(8):
                            wsl = slice((kt * NI + i) * 128, (kt * NI + i + 1) * 128)
                            nc.tensor.matmul(ps_x[:], winx[:, wsl],
                                             xck[:, kt * HTC:(kt + 1) * HTC],
                                             start=(kt == 0), stop=(kt == 7))
                        nc.scalar.copy(xin[:, 3 + h * HTC:3 + (h + 1) * HTC], ps_x[:])
                    nc.gpsimd.tensor_copy(xtail[:, i * 3:i * 3 + 3], xin[:, TC:TC + 3])

                    # -- causal conv from padded SBUF staging (Pool)
                    acc = stg.tile([128, TC], F32, tag="acc")
                    u = stg.tile([128, TC], F32, tag="u")
                    nc.vector.tensor_scalar_mul(acc[:], xin[:, 0:TC], cw[:, i * KC:i * KC + 1])
                    nc.vector.scalar_tensor_tensor(
                        out=u[:], in0=xin[:, 1:1 + TC], scalar=cw[:, i * KC + 1:i * KC + 2],
                        in1=acc[:], op0=OP.mult, op1=OP.add)
                    nc.vector.scalar_tensor_tensor(
                        out=acc[:], in0=xin[:, 2:2 + TC], scalar=cw[:, i * KC + 2:i * KC + 3],
                        in1=u[:], op0=OP.mult, op1=OP.add)
                    nc.vector.scalar_tensor_tensor(
                        out=u[:], in0=xin[:, 3:3 + TC], scalar=cw[:, i * KC + 3:i * KC + 4],
                        in1=acc[:], op0=OP.mult, op1=OP.add)

                    # -- xs = silu(u + cb): sigmoid via Exp + fast reciprocal
                    e1 = wk.tile([128, TC], F32, tag="e")
                    nc.scalar.activation(e1[:], u[:], AF.Exp,
                                         bias=ncb[:, i:i + 1], scale=-1.0)
                    p1 = wk.tile([128, TC], F32, tag="e")
                    nc.vector.tensor_scalar_add(p1[:], e1[:], 1.0)
                    rc = wk.tile([128, TC], F32, tag="e")
                    nc.vector.reciprocal_approx_fast(rc[:], p1[:])
                    xs = kp.tile([128, TC], BF16, tag="xs")
                    nc.vector.scalar_tensor_tensor(
                        out=xs[:], in0=u[:], scalar=cb[:, i:i + 1], in1=rc[:],
                        op0=OP.add, op1=OP.mult)
                    xs_i.append(xs)

                    # -- z branch + gate g = z*sigmoid(z), per half from PSUM
                    g = kp.tile([128, TC], BF16, tag="g")
                    for h in range(2):
                        xck = ctx[(c, "xck", h)]
                        ps_z = psin.tile([128, HTC], F32, tag="ps")
                        for kt in range(8):
                            wsl = slice((kt * NI + i) * 128, (kt * NI + i + 1) * 128)
                            nc.tensor.matmul(ps_z[:], winz[:, wsl],
                                             xck[:, kt * HTC:(kt + 1) * HTC],
                                             start=(kt == 0), stop=(kt == 7))
                        e2 = wk.tile([128, HTC], F32, tag="eh")
                        nc.scalar.activation(e2[:], ps_z[:], AF.Exp, scale=-1.0)
                        p2 = wk.tile([128, HTC], F32, tag="eh")
                        nc.vector.tensor_scalar_add(p2[:], e2[:], 1.0)
                        rz = wk.tile([128, HTC], F32, tag="eh")
                        nc.vector.reciprocal_approx_fast(rz[:], p2[:])
                        nc.vector.tensor_tensor(out=g[:, h * HTC:(h + 1) * HTC],
                                                in0=ps_z[:], in1=rz[:], op=OP.mult)
                    g_i.append(g)

                    # -- x_dbl partial (accumulate over i, per half)
                    for h in range(2):
                        if i == 0:
                            psxd_t = psxd.tile([R + 2 * S, HTC], F32, tag="xd")
                            ps_xd[h] = psxd_t
                        nc.tensor.matmul(ps_xd[h][:],
                                         wxp[:, i * (R + 2 * S):(i + 1) * (R + 2 * S)],
                                         xs[:, h * HTC:(h + 1) * HTC],
                                         start=(i == 0), stop=(i == NI - 1))

                xd_sb = wk1.tile([R + 2 * S, TC], BF16, tag="xdsb")
                for h in range(2):
                    nc.scalar.copy(xd_sb[:, h * HTC:(h + 1) * HTC], ps_xd[h][:])
                xd_part = dr.tile([R + 2 * S, TC], BF16, tag="xdp")
                nc.sync.dma_start(xd_part[:], xd_sb[:])
                xd_red = nc.dram_tensor(f"xd_red_{c}", [R + 2 * S, TC], BF16)
                nc.gpsimd.collective_compute(
                    "AllReduce", OP.add, replica_groups=GROUPS,
                    ins=[xd_part[:]], outs=[xd_red.ap()])
                ctx[c] = dict(xs_i=xs_i, g_i=g_i, xd_red=xd_red)

            def mid(c):
                """dt, B/C broadcast, scans, reduce, gate, out partial, RS."""
                xs_i = ctx[c]["xs_i"]
                g_i = ctx[c]["g_i"]
                xd_red = ctx[c]["xd_red"]

                dtr = wk1.tile([R, TC], BF16, tag="dtr")
                nc.sync.dma_start(dtr[:], xd_red.ap()[0:R, :])

                dt_i, dtx_i, yac_i = [], [], []
                for i in range(NI):
                    dt = mk.tile([128, TC], BF16, tag="dt")
                    for h in range(2):
                        ps_dt = psdt.tile([128, HTC], F32, tag="psd")
                        nc.tensor.matmul(ps_dt[:], wdt[:, i * 128:(i + 1) * 128],
                                         dtr[:, h * HTC:(h + 1) * HTC],
                                         start=True, stop=True)
                        edt = wk1.tile([128, HTC], F32, tag="edt")
                        nc.scalar.activation(edt[:], ps_dt[:], AF.Exp,
                                             bias=bdt[:, i:i + 1])
                        nc.scalar.activation(dt[:, h * HTC:(h + 1) * HTC], edt[:],
                                             AF.Ln, bias=1.0)
                    dt_i.append(dt)
                    dtx = mk.tile([128, TC], BF16, tag="dtx")
                    nc.vector.tensor_tensor(out=dtx[:], in0=dt[:], in1=xs_i[i][:],
                                            op=OP.mult)
                    dtx_i.append(dtx)
                    yac_i.append(None)

                for sg in range(NSG):
                    s0 = sg * SG
                    b_bc = bcp.tile([128, SG * TC], BF16, tag="bbc")
                    c_bc = bcp.tile([128, SG * TC], BF16, tag="cbc")
                    for j in range(SG):
                        nc.sync.dma_start(
                            b_bc[:, j * TC:(j + 1) * TC],
                            xd_red.ap()[R + s0 + j:R + s0 + j + 1, :].broadcast_to([128, TC]))
                        nc.sync.dma_start(
                            c_bc[:, j * TC:(j + 1) * TC],
                            xd_red.ap()[R + S + s0 + j:R + S + s0 + j + 1, :].broadcast_to([128, TC]))
                    for i in range(NI):
                        a_t = sca.tile([128, SG * TC], BF16, tag="a_t")
                        for j in range(SG):
                            nc.scalar.activation(
                                a_t[:, j * TC:(j + 1) * TC], dt_i[i][:], AF.Exp,
                                scale=am[:, i * S + s0 + j:i * S + s0 + j + 1])
                        bb = sc.tile([128, SG * TC], BF16, tag="bb")
                        nc.vector.tensor_tensor(
                            out=bb[:].rearrange("p (s t) -> p s t", s=SG),
                            in0=dtx_i[i][:].unsqueeze(1).broadcast_to([128, SG, TC]),
                            in1=b_bc[:].rearrange("p (s t) -> p s t", s=SG), op=OP.mult)

                        av = a_t[:].rearrange("p (s t) -> p s t", s=SG)
                        hoff = (i * NSG + sg) * SG
                        if c > 0:
                            bv = bb[:].rearrange("p (s t) -> p s t", s=SG)
                            ahead = wk.tile([128, SG], BF16, tag="ahead")
                            nc.gpsimd.tensor_copy(ahead[:], av[:, :, 0])
                            tmp = wk.tile([128, SG], BF16, tag="tmpf")
                            nc.vector.tensor_tensor(out=tmp[:], in0=ahead[:],
                                                    in1=hprev[:, hoff:hoff + SG], op=OP.mult)
                            nc.vector.tensor_tensor(out=bv[:, :, 0], in0=bv[:, :, 0],
                                                    in1=tmp[:], op=OP.add)
                        nc.gpsimd.memset(av[:, :, 0], 0.0)

                        h_t = sc.tile([128, SG * TC], BF16, tag="h_t")
                        if sg in POOL_SCAN_SG:
                            nc.gpsimd.tensor_tensor_scan(h_t[:], a_t[:], bb[:], 0.0,
                                                         op0=OP.mult, op1=OP.add)
                        else:
                            nc.vector.tensor_tensor_scan(h_t[:], a_t[:], bb[:], 0.0,
                                                         op0=OP.mult, op1=OP.add)
                        if c < NCH - 1:
                            nc.gpsimd.tensor_copy(
                                hprev[:, hoff:hoff + SG],
                                h_t[:].rearrange("p (s t) -> p s t", s=SG)[:, :, TC - 1])

                        # hc -> bb (reuse), tree halves -> h_t slices (reuse)
                        nc.vector.tensor_tensor(out=bb[:], in0=h_t[:], in1=c_bc[:],
                                                op=OP.mult)
                        nc.gpsimd.tensor_tensor(out=h_t[:, 0:SG * TC // 2],
                                                in0=bb[:, 0:SG * TC // 2],
                                                in1=bb[:, SG * TC // 2:], op=OP.add)
                        if sg == 0:
                            yac = mk.tile([128, TC], BF16, tag="yac")
                            nc.vector.tensor_tensor(out=yac[:], in0=h_t[:, 0:TC],
                                                    in1=h_t[:, TC:2 * TC], op=OP.add)
                            yac_i[i] = yac
                        else:
                            r2 = wk1.tile([128, TC], BF16, tag="r2")
                            nc.vector.tensor_tensor(out=r2[:], in0=h_t[:, 0:TC],
                                                    in1=h_t[:, TC:2 * TC], op=OP.add)
                            nc.vector.tensor_tensor(out=yac_i[i][:], in0=yac_i[i][:],
                                                    in1=r2[:], op=OP.add)

                yg_i = []
                for i in range(NI):
                    yD = wk1.tile([128, TC], BF16, tag="yD")
                    nc.vector.scalar_tensor_tensor(
                        out=yD[:], in0=xs_i[i][:], scalar=dv[:, i:i + 1],
                        in1=yac_i[i][:], op0=OP.mult, op1=OP.add)
                    yg = ygp.tile([128, TC], BF16, tag="yg")
                    nc.vector.tensor_tensor(out=yg[:], in0=yD[:], in1=g_i[i][:],
                                            op=OP.mult)
                    yg_i.append(yg)

                yo_part = dr.tile([DM, TC], BF16, tag="yop")
                for mt in range(8):
                    for h in range(2):
                        ps_o = pso.tile([128, HTC], F32, tag="pso")
                        for i in range(NI):
                            nc.tensor.matmul(ps_o[:],
                                             wc[:, (i * 8 + mt) * 128:(i * 8 + mt + 1) * 128],
                                             yg_i[i][:, h * HTC:(h + 1) * HTC],
                                             start=(i == 0), stop=(i == NI - 1))
                        yo_sb = op_.tile([128, HTC], BF16, tag="yos")
                        nc.scalar.copy(yo_sb[:], ps_o[:])
                        nc.sync.dma_start(
                            yo_part[mt * 128:(mt + 1) * 128, h * HTC:(h + 1) * HTC],
                            yo_sb[:])
                yo_red = nc.dram_tensor(f"yo_red_{c}", [EL, TC], BF16)
                nc.gpsimd.collective_compute(
                    "ReduceScatter", OP.add, replica_groups=GROUPS,
                    ins=[yo_part[:]], outs=[yo_red.ap()])
                ctx[c]["yo_red"] = yo_red

            def tail(c):
                """RS result + bias -> out."""
                yo_red = ctx[c]["yo_red"]
                t0 = c * TC
                for mt in range(NE):
                    yin = op_.tile([128, TC], BF16, tag="yin")
                    nc.sync.dma_start(yin[:], yo_red.ap()[mt * 128:(mt + 1) * 128, :])
                    o = op_.tile([128, TC], F32, tag="o")
                    nc.scalar.activation(o[:], yin[:], AF.Identity, bias=bo[:, mt:mt + 1])
                    nc.sync.dma_start(out[mt * 128:(mt + 1) * 128, t0:t0 + TC], o[:])

            for c in range(NCH + 2):
                if c < NCH:
                    front(c)
                if 1 <= c <= NCH:
                    mid(c - 1)
                if c >= 2:
                    tail(c - 2)

    nc.compile()
    _NC_CACHE["nc"] = nc
    return nc


def _prep_inputs(inputs):
    x = np.ascontiguousarray(np.asarray(inputs["x"], np.float32))
    W_in = np.asarray(inputs["W_in"], np.float32)
    conv_w = np.asarray(inputs["conv_w"], np.float32)
    conv_b = np.asarray(inputs["conv_b"], np.float32)
    W_xp = np.asarray(inputs["W_xp"], np.float32)
    W_dt = np.asarray(inputs["W_dt"], np.float32)
    b_dt = np.asarray(inputs["b_dt"], np.float32)
    A_log = np.asarray(inputs["A_log"], np.float32)
    D = np.asarray(inputs["D"], np.float32)
    W_out_ssm = np.asarray(inputs["W_out_ssm"], np.float32)
    W_out = np.asarray(inputs["W_out"], np.float32)
    b_out = np.asarray(inputs["b_out"], np.float32)

    A = -np.exp(A_log)
    W_c = (W_out.astype(np.float64) @ W_out_ssm.astype(np.float64)).astype(np.float32)

    in_maps = []
    for k in range(NC):
        b, r = divmod(k, TPG)
        dsl = slice(r * DIL, (r + 1) * DIL)
        esl = slice(r * EL, (r + 1) * EL)
        in_maps.append({
            "x_t": np.ascontiguousarray(x[b].T.astype(BFnp)),
            "w_in_x": np.ascontiguousarray(W_in[dsl, :].T.astype(BFnp)),
            "w_in_z": np.ascontiguousarray(
                W_in[DI + r * DIL: DI + (r + 1) * DIL, :].T.astype(BFnp)),
            "conv_w": np.ascontiguousarray(conv_w[dsl, 0, :]),
            "conv_b": np.ascontiguousarray(conv_b[dsl][:, None]),
            "ncb_in": np.ascontiguousarray(-conv_b[dsl][:, None]),
            "w_xp": np.ascontiguousarray(W_xp[:, dsl].T.astype(BFnp)),
            "w_dt": np.ascontiguousarray(W_dt[dsl, :].T.astype(BFnp)),
            "b_dt": np.ascontiguousarray(b_dt[dsl][:, None]),
            "a_mat": np.ascontiguousarray(A[dsl, :]),
            "d_vec": np.ascontiguousarray(D[dsl][:, None]),
            "w_c": np.ascontiguousarray(W_c[:, dsl].T.astype(BFnp)),
            "b_o": np.ascontiguousarray(b_out[esl][:, None]),
        })
    return in_maps


def _assemble(results):
    full = np.zeros((B, L, DM), np.float32)
    for k in range(NC):
        b, r = divmod(k, TPG)
        full[b, :, r * EL:(r + 1) * EL] = results[k]["out"].T
    return full


def kernel(**inputs):
    nc = build()
    in_maps = _prep_inputs(inputs)
    res = run_bass_kernel_spmd(nc, in_maps, core_ids=list(range(NC)))
    return _assemble(res.results)


def kernel_sim(**inputs):
    """Run through MultiCoreSim instead of HW (for debugging)."""
    from concourse.bass_interp import MultiCoreSim
    nc = build()
    in_maps = _prep_inputs(inputs)
    sim = MultiCoreSim(nc, num_cores=NC)
    for k in range(NC):
        for name, arr in in_maps[k].items():
            sim.cores[k].tensor(name)[:] = arr
    sim.simulate(check_with_hw=False)
    results = [{"out": sim.cores[k].tensor("out").copy()} for k in range(NC)]
    return _assemble(results)


# revision 12
# speedup vs baseline: 1.5472x; 1.0628x over previous
"""Mamba SSM block on 8 TRN2 NeuronCores (Bass/Tile, SPMD).

Sharding: 2-way batch DP x 4-way d_inner TP (512 channels/core, each core
processes only its batch's 2048 tokens in 2 chunks of 1024).

Per chunk: in_proj + causal conv + silu (sigmoid via exp + DVE fast
reciprocal, keeping a single ACT table: natural_log_exp), x_dbl partial +
AllReduce [96,1024] over the 4-core TP group. Scan phase: dt via
exp/ln softplus, per 4-state group: B/C row-broadcasts via DMA
(stride-0 partition replication), a_t = exp(A*dt) on ACT, bb on DVE 2x,
one merged tensor_tensor_scan per group (a=0 at segment heads folds the
per-state initial condition into the first bb element), h*C and a 2x
binary-tree reduce on DVE, with one state-group's scan on the Pool
engine. out_proj computes a full d_model partial per core (W_out @
W_out_ssm folded host-side) and a ReduceScatter replaces the baseline's
AllGather. All matmuls n=512 to keep 8 PSUM banks free-flowing.
"""
import numpy as np
import ml_dtypes

import concourse.bass as bass
import concourse.tile as tile
from concourse import bacc, mybir
from concourse.bass_utils import run_bass_kernel_spmd

BFnp = ml_dtypes.bfloat16
F32 = mybir.dt.float32
BF16 = mybir.dt.bfloat16
AF = mybir.ActivationFunctionType
OP = mybir.AluOpType

NC = 8
TPG = 4                   # tensor-parallel group size
B, L, DM = 2, 2048, 1024
DI, S, R, KC = 2048, 16, 64, 4
DIL = DI // TPG           # 512 d_inner per core
NI = DIL // 128           # 4 partition tiles
TC = 1024                 # tokens per chunk
NCH = L // TC             # 2 chunks (each core: only its batch)
SG = 4                    # states per scan group
NSG = S // SG             # 4 groups
EL = DM // TPG            # 256 output rows per core
NE = EL // 128
HTC = TC // 2             # matmul n (half chunk)
GROUPS = [[0, 1, 2, 3], [4, 5, 6, 7]]

POOL_SCAN_SG = set()      # Pool rejects TensorScalarPtr/scan at codegen

_NC_CACHE = {}


def build():
    if "nc" in _NC_CACHE:
        return _NC_CACHE["nc"]
    nc = bacc.Bacc("TRN2", target_bir_lowering=False, debug=False, num_devices=NC)

    # ---- per-core DRAM inputs (host pre-sharded / transposed / casted) ----
    x_t = nc.dram_tensor("x_t", [DM, L], BF16, kind="ExternalInput")       # own batch
    w_in_x = nc.dram_tensor("w_in_x", [DM, DIL], BF16, kind="ExternalInput")
    w_in_z = nc.dram_tensor("w_in_z", [DM, DIL], BF16, kind="ExternalInput")
    conv_w = nc.dram_tensor("conv_w", [DIL, KC], F32, kind="ExternalInput")
    conv_b = nc.dram_tensor("conv_b", [DIL, 1], F32, kind="ExternalInput")
    ncb_in = nc.dram_tensor("ncb_in", [DIL, 1], F32, kind="ExternalInput")  # -conv_b
    w_xp = nc.dram_tensor("w_xp", [DIL, R + 2 * S], BF16, kind="ExternalInput")
    w_dt = nc.dram_tensor("w_dt", [R, DIL], BF16, kind="ExternalInput")
    b_dt = nc.dram_tensor("b_dt", [DIL, 1], F32, kind="ExternalInput")
    a_mat = nc.dram_tensor("a_mat", [DIL, S], F32, kind="ExternalInput")   # -exp(A_log)
    d_vec = nc.dram_tensor("d_vec", [DIL, 1], F32, kind="ExternalInput")
    w_c = nc.dram_tensor("w_c", [DIL, DM], BF16, kind="ExternalInput")     # W_c[:,ch].T
    b_o = nc.dram_tensor("b_o", [EL, 1], F32, kind="ExternalInput")
    out = nc.dram_tensor("out", [EL, L], F32, kind="ExternalOutput")

    with tile.TileContext(nc) as tc:
        with (
            tc.tile_pool(name="wpool", bufs=1) as wp,     # persistent weights
            tc.tile_pool(name="xpool", bufs=2) as xp,     # x half-chunk stream
            tc.tile_pool(name="stage", bufs=1) as stg,    # xin / u / acc staging
            tc.tile_pool(name="work", bufs=2) as wk,      # silu chains etc.
            tc.tile_pool(name="work1", bufs=1) as wk1,    # single-buffered transients
            tc.tile_pool(name="keep", bufs=6) as kp,      # xs/g live across phases
            tc.tile_pool(name="midk", bufs=NI) as mk,     # dt/dtx/y_acc per chunk
            tc.tile_pool(name="scan", bufs=1) as sc,      # bb/h tiles
            tc.tile_pool(name="scana", bufs=2) as sca,    # a (ACT-written, dbl)
            tc.tile_pool(name="bcast", bufs=2) as bcp,    # B/C broadcast tiles
            tc.tile_pool(name="outp", bufs=2) as op_,     # yo tiles
            tc.tile_pool(name="ygp", bufs=NI) as ygp,     # yg (4 live per chunk)
            tc.tile_pool(name="state", bufs=1) as st,     # hprev / xtail
            tc.tile_pool(name="psin", bufs=2, space="PSUM") as psin,
            tc.tile_pool(name="psxd", bufs=2, space="PSUM") as psxd,
            tc.tile_pool(name="psdt", bufs=2, space="PSUM") as psdt,
            tc.tile_pool(name="pso", bufs=2, space="PSUM") as pso,
            tc.tile_pool(name="dram", bufs=2, space="DRAM") as dr,
        ):
            # ---------- load weights ----------
            winx = wp.tile([128, 8 * NI * 128], BF16, tag="winx")
            winz = wp.tile([128, 8 * NI * 128], BF16, tag="winz")
            for kt in range(8):
                for i in range(NI):
                    csl = slice((kt * NI + i) * 128, (kt * NI + i + 1) * 128)
                    nc.sync.dma_start(winx[:, csl],
                                      w_in_x[kt * 128:(kt + 1) * 128, i * 128:(i + 1) * 128])
                    nc.sync.dma_start(winz[:, csl],
                                      w_in_z[kt * 128:(kt + 1) * 128, i * 128:(i + 1) * 128])
            wxp = wp.tile([128, NI * (R + 2 * S)], BF16, tag="wxp")
            for i in range(NI):
                nc.sync.dma_start(wxp[:, i * (R + 2 * S):(i + 1) * (R + 2 * S)],
                                  w_xp[i * 128:(i + 1) * 128, :])
            wdt = wp.tile([R, DIL], BF16, tag="wdt")
            nc.sync.dma_start(wdt[:], w_dt[:, :])
            wc = wp.tile([128, NI * 8 * 128], BF16, tag="wc")
            for i in range(NI):
                for mt in range(8):
                    nc.sync.dma_start(
                        wc[:, (i * 8 + mt) * 128:(i * 8 + mt + 1) * 128],
                        w_c[i * 128:(i + 1) * 128, mt * 128:(mt + 1) * 128])
            cw = wp.tile([128, NI * KC], F32, tag="cw")
            cb = wp.tile([128, NI], F32, tag="cb")
            ncb = wp.tile([128, NI], F32, tag="ncb")
            bdt = wp.tile([128, NI], F32, tag="bdt")
            dv = wp.tile([128, NI], F32, tag="dv")
            am = wp.tile([128, NI * S], F32, tag="am")
            for i in range(NI):
                sl = slice(i * 128, (i + 1) * 128)
                nc.sync.dma_start(cw[:, i * KC:(i + 1) * KC], conv_w[sl, :])
                nc.sync.dma_start(cb[:, i:i + 1], conv_b[sl, :])
                nc.sync.dma_start(ncb[:, i:i + 1], ncb_in[sl, :])
                nc.sync.dma_start(bdt[:, i:i + 1], b_dt[sl, :])
                nc.sync.dma_start(dv[:, i:i + 1], d_vec[sl, :])
                nc.sync.dma_start(am[:, i * S:(i + 1) * S], a_mat[sl, :])
            bo = wp.tile([128, NE], F32, tag="bo")
            for mt in range(NE):
                nc.sync.dma_start(bo[:, mt:mt + 1], b_o[mt * 128:(mt + 1) * 128, :])

            hprev = st.tile([128, NI * S], BF16, tag="hprev")
            xtail = st.tile([128, NI * 3], BF16, tag="xtail")

            ctx = {}

            def front(c):
                """x stream, in_proj, conv, silu, z-gate, x_dbl partial, AR."""
                t0 = c * TC
                xs_i, g_i = [], []
                ps_xd = [None, None]
                for i in range(NI):
                    # -- in_proj x branch, both halves -> xin staging (f32)
                    xin = stg.tile([128, TC + 3], BF16, tag="xin")
                    if c == 0:
                        nc.gpsimd.memset(xin[:, 0:3], 0.0)
                    else:
                        nc.gpsimd.tensor_copy(xin[:, 0:3], xtail[:, i * 3:i * 3 + 3])
                    for h in range(2):
                        xck = xp.tile([128, 8 * HTC], BF16, tag="xck")
                        if i == 0:
                            for kt in range(8):
                                nc.sync.dma_start(
                                    xck[:, kt * HTC:(kt + 1) * HTC],
                                    x_t[kt * 128:(kt + 1) * 128,
                                        t0 + h * HTC:t0 + (h + 1) * HTC])
                            ctx[(c, "xck", h)] = xck
                        else:
                            xck = ctx[(c, "xck", h)]
                        ps_x = psin.tile([128, HTC], F32, tag="ps")
                        for kt in range(8):
                            wsl = slice((kt * NI + i) * 128, (kt * NI + i + 1) * 128)
                            nc.tensor.matmul(ps_x[:], winx[:, wsl],
                                             xck[:, kt * HTC:(kt + 1) * HTC],
                                             start=(kt == 0), stop=(kt == 7))
                        nc.scalar.copy(xin[:, 3 + h * HTC:3 + (h + 1) * HTC], ps_x[:])
                    nc.gpsimd.tensor_copy(xtail[:, i * 3:i * 3 + 3], xin[:, TC:TC + 3])

                    # -- causal conv from padded SBUF staging (Pool)
                    acc = stg.tile([128, TC], F32, tag="acc")
                    u = stg.tile([128, TC], F32, tag="u")
                    nc.vector.tensor_scalar_mul(acc[:], xin[:, 0:TC], cw[:, i * KC:i * KC + 1])
                    nc.vector.scalar_tensor_tensor(
                        out=u[:], in0=xin[:, 1:1 + TC], scalar=cw[:, i * KC + 1:i * KC + 2],
                        in1=acc[:], op0=OP.mult, op1=OP.add)
                    nc.vector.scalar_tensor_tensor(
                        out=acc[:], in0=xin[:, 2:2 + TC], scalar=cw[:, i * KC + 2:i * KC + 3],
                        in1=u[:], op0=OP.mult, op1=OP.add)
                    nc.vector.scalar_tensor_tensor(
                        out=u[:], in0=xin[:, 3:3 + TC], scalar=cw[:, i * KC + 3:i * KC + 4],
                        in1=acc[:], op0=OP.mult, op1=OP.add)

                    # -- xs = silu(u + cb): sigmoid via Exp + fast reciprocal
                    e1 = wk.tile([128, TC], F32, tag="e")
                    nc.scalar.activation(e1[:], u[:], AF.Exp,
                                         bias=ncb[:, i:i + 1], scale=-1.0)
                    p1 = wk.tile([128, TC], F32, tag="e")
                    nc.vector.tensor_scalar_add(p1[:], e1[:], 1.0)
                    rc = wk.tile([128, TC], F32, tag="e")
                    nc.vector.reciprocal_approx_fast(rc[:], p1[:])
                    xs = kp.tile([128, TC], BF16, tag="xs")
                    nc.vector.scalar_tensor_tensor(
                        out=xs[:], in0=u[:], scalar=cb[:, i:i + 1], in1=rc[:],
                        op0=OP.add, op1=OP.mult)
                    xs_i.append(xs)

                    # -- z branch + gate g = z*sigmoid(z), per half from PSUM
                    g = kp.tile([128, TC], BF16, tag="g")
                    for h in range(2):
                        xck = ctx[(c, "xck", h)]
                        ps_z = psin.tile([128, HTC], F32, tag="ps")
                        for kt in range(8):
                            wsl = slice((kt * NI + i) * 128, (kt * NI + i + 1) * 128)
                            nc.tensor.matmul(ps_z[:], winz[:, wsl],
                                             xck[:, kt * HTC:(kt + 1) * HTC],
                                             start=(kt == 0), stop=(kt == 7))
                        e2 = wk.tile([128, HTC], F32, tag="eh")
                        nc.scalar.activation(e2[:], ps_z[:], AF.Exp, scale=-1.0)
                        p2 = wk.tile([128, HTC], F32, tag="eh")
                        nc.vector.tensor_scalar_add(p2[:], e2[:], 1.0)
                        rz = wk.tile([128, HTC], F32, tag="eh")
                        nc.vector.reciprocal_approx_fast(rz[:], p2[:])
                        nc.vector.tensor_tensor(out=g[:, h * HTC:(h + 1) * HTC],
                                                in0=ps_z[:], in1=rz[:], op=OP.mult)
                    g_i.append(g)

                    # -- x_dbl partial (accumulate over i, per half)
                    for h in range(2):
                        if i == 0:
                            psxd_t = psxd.tile([R + 2 * S, HTC], F32, tag="xd")
                            ps_xd[h] = psxd_t
                        nc.tensor.matmul(ps_xd[h][:],
                                         wxp[:, i * (R + 2 * S):(i + 1) * (R + 2 * S)],
                                         xs[:, h * HTC:(h + 1) * HTC],
                                         start=(i == 0), stop=(i == NI - 1))

                xd_sb = wk1.tile([R + 2 * S, TC], BF16, tag="xdsb")
                for h in range(2):
                    nc.scalar.copy(xd_sb[:, h * HTC:(h + 1) * HTC], ps_xd[h][:])
                xd_part = dr.tile([R + 2 * S, TC], BF16, tag="xdp")
                nc.sync.dma_start(xd_part[:], xd_sb[:])
                xd_red = nc.dram_tensor(f"xd_red_{c}", [R + 2 * S, TC], BF16)
                nc.gpsimd.collective_compute(
                    "AllReduce", OP.add, replica_groups=GROUPS,
                    ins=[xd_part[:]], outs=[xd_red.ap()])
                ctx[c] = dict(xs_i=xs_i, g_i=g_i, xd_red=xd_red)

            def mid(c):
                """dt, B/C broadcast, scans, reduce, gate, out partial, RS."""
                xs_i = ctx[c]["xs_i"]
                g_i = ctx[c]["g_i"]
                xd_red = ctx[c]["xd_red"]

                dtr = wk1.tile([R, TC], BF16, tag="dtr")
                nc.sync.dma_start(dtr[:], xd_red.ap()[0:R, :])

                dt_i, dtx_i, yac_i = [], [], []
                for i in range(NI):
                    dt = mk.tile([128, TC], BF16, tag="dt")
                    for h in range(2):
                        ps_dt = psdt.tile([128, HTC], F32, tag="psd")
                        nc.tensor.matmul(ps_dt[:], wdt[:, i * 128:(i + 1) * 128],
                                         dtr[:, h * HTC:(h + 1) * HTC],
                                         start=True, stop=True)
                        edt = wk1.tile([128, HTC], F32, tag="edt")
                        nc.scalar.activation(edt[:], ps_dt[:], AF.Exp,
                                             bias=bdt[:, i:i + 1])
                        nc.scalar.activation(dt[:, h * HTC:(h + 1) * HTC], edt[:],
                                             AF.Ln, bias=1.0)
                    dt_i.append(dt)
                    dtx = mk.tile([128, TC], BF16, tag="dtx")
                    nc.vector.tensor_tensor(out=dtx[:], in0=dt[:], in1=xs_i[i][:],
                                            op=OP.mult)
                    dtx_i.append(dtx)
                    yac_i.append(None)

                for sg in range(NSG):
                    s0 = sg * SG
                    b_bc = bcp.tile([128, SG * TC], BF16, tag="bbc")
                    c_bc = bcp.tile([128, SG * TC], BF16, tag="cbc")
                    for j in range(SG):
                        nc.sync.dma_start(
                            b_bc[:, j * TC:(j + 1) * TC],
                            xd_red.ap()[R + s0 + j:R + s0 + j + 1, :].broadcast_to([128, TC]))
                        nc.sync.dma_start(
                            c_bc[:, j * TC:(j + 1) * TC],
                            xd_red.ap()[R + S + s0 + j:R + S + s0 + j + 1, :].broadcast_to([128, TC]))
                    for i in range(NI):
                        a_t = sca.tile([128, SG * TC], BF16, tag="a_t")
                        for j in range(SG):
                            nc.scalar.activation(
                                a_t[:, j * TC:(j + 1) * TC], dt_i[i][:], AF.Exp,
                                scale=am[:, i * S + s0 + j:i * S + s0 + j + 1])
                        bb = sc.tile([128, SG * TC], BF16, tag="bb")
                        for j in range(SG):
                            nc.vector.tensor_tensor(
                                out=bb[:, j * TC:(j + 1) * TC], in0=dtx_i[i][:],
                                in1=b_bc[:, j * TC:(j + 1) * TC], op=OP.mult)

                        av = a_t[:].rearrange("p (s t) -> p s t", s=SG)
                        hoff = (i * NSG + sg) * SG
                        if c > 0:
                            bv = bb[:].rearrange("p (s t) -> p s t", s=SG)
                            ahead = wk.tile([128, SG], BF16, tag="ahead")
                            nc.vector.tensor_copy(ahead[:], av[:, :, 0])
                            tmp = wk.tile([128, SG], BF16, tag="tmpf")
                            nc.vector.tensor_tensor(out=tmp[:], in0=ahead[:],
                                                    in1=hprev[:, hoff:hoff + SG], op=OP.mult)
                            nc.vector.tensor_tensor(out=bv[:, :, 0], in0=bv[:, :, 0],
                                                    in1=tmp[:], op=OP.add)
                        nc.vector.memset(av[:, :, 0], 0.0)

                        h_t = sc.tile([128, SG * TC], BF16, tag="h_t")
                        if sg in POOL_SCAN_SG:
                            nc.gpsimd.tensor_tensor_scan(h_t[:], a_t[:], bb[:], 0.0,
                                                         op0=OP.mult, op1=OP.add)
                        else:
                            nc.vector.tensor_tensor_scan(h_t[:], a_t[:], bb[:], 0.0,
                                                         op0=OP.mult, op1=OP.add)
                        if c < NCH - 1:
                            nc.gpsimd.tensor_copy(
                                hprev[:, hoff:hoff + SG],
                                h_t[:].rearrange("p (s t) -> p s t", s=SG)[:, :, TC - 1])

                        # hc -> bb (reuse), tree halves -> h_t slices (reuse)
                        nc.vector.tensor_tensor(out=bb[:], in0=h_t[:], in1=c_bc[:],
                                                op=OP.mult)
                        nc.vector.tensor_tensor(out=h_t[:, 0:SG * TC // 2],
                                                in0=bb[:, 0:SG * TC // 2],
                                                in1=bb[:, SG * TC // 2:], op=OP.add)
                        if sg == 0:
                            yac = mk.tile([128, TC], BF16, tag="yac")
                            nc.vector.tensor_tensor(out=yac[:], in0=h_t[:, 0:TC],
                                                    in1=h_t[:, TC:2 * TC], op=OP.add)
                            yac_i[i] = yac
                        else:
                            nc.vector.tensor_tensor(out=h_t[:, 2 * TC:3 * TC],
                                                    in0=h_t[:, 0:TC],
                                                    in1=h_t[:, TC:2 * TC], op=OP.add)
                            nc.vector.tensor_tensor(out=yac_i[i][:], in0=yac_i[i][:],
                                                    in1=h_t[:, 2 * TC:3 * TC], op=OP.add)

                yg_i = []
                for i in range(NI):
                    yD = wk1.tile([128, TC], BF16, tag="yD")
                    nc.vector.scalar_tensor_tensor(
                        out=yD[:], in0=xs_i[i][:], scalar=dv[:, i:i + 1],
                        in1=yac_i[i][:], op0=OP.mult, op1=OP.add)
                    yg = ygp.tile([128, TC], BF16, tag="yg")
                    nc.vector.tensor_tensor(out=yg[:], in0=yD[:], in1=g_i[i][:],
                                            op=OP.mult)
                    yg_i.append(yg)

                yo_part = dr.tile([DM, TC], BF16, tag="yop")
                for mt in range(8):
                    for h in range(2):
                        ps_o = pso.tile([128, HTC], F32, tag="pso")
                        for i in range(NI):
                            nc.tensor.matmul(ps_o[:],
                                             wc[:, (i * 8 + mt) * 128:(i * 8 + mt + 1) * 128],
                                             yg_i[i][:, h * HTC:(h + 1) * HTC],
                                             start=(i == 0), stop=(i == NI - 1))
                        yo_sb = op_.tile([128, HTC], BF16, tag="yos")
                        nc.scalar.copy(yo_sb[:], ps_o[:])
                        nc.sync.dma_start(
                            yo_part[mt * 128:(mt + 1) * 128, h * HTC:(h + 1) * HTC],
                            yo_sb[:])
                yo_red = nc.dram_tensor(f"yo_red_{c}", [EL, TC], BF16)
                nc.gpsimd.collective_compute(
                    "ReduceScatter", OP.add, replica_groups=GROUPS,
                    ins=[yo_part[:]], outs=[yo_red.ap()])
                ctx[c]["yo_red"] = yo_red

            def tail(c):
                """RS result + bias -> out."""
                yo_red = ctx[c]["yo_red"]
                t0 = c * TC
                for mt in range(NE):
                    yin = op_.tile([128, TC], BF16, tag="yin")
                    nc.sync.dma_start(yin[:], yo_red.ap()[mt * 128:(mt + 1) * 128, :])
                    o = op_.tile([128, TC], F32, tag="o")
                    nc.scalar.activation(o[:], yin[:], AF.Identity, bias=bo[:, mt:mt + 1])
                    nc.sync.dma_start(out[mt * 128:(mt + 1) * 128, t0:t0 + TC], o[:])

            for c in range(NCH + 2):
                if c < NCH:
                    front(c)
                if 1 <= c <= NCH:
                    mid(c - 1)
                if c >= 2:
                    tail(c - 2)

    nc.compile()
    _NC_CACHE["nc"] = nc
    return nc


def _prep_inputs(inputs):
    x = np.ascontiguousarray(np.asarray(inputs["x"], np.float32))
    W_in = np.asarray(inputs["W_in"], np.float32)
    conv_w = np.asarray(inputs["conv_w"], np.float32)
    conv_b = np.asarray(inputs["conv_b"], np.float32)
    W_xp = np.asarray(inputs["W_xp"], np.float32)
    W_dt = np.asarray(inputs["W_dt"], np.float32)
    b_dt = np.asarray(inputs["b_dt"], np.float32)
    A_log = np.asarray(inputs["A_log"], np.float32)
    D = np.asarray(inputs["D"], np.float32)
    W_out_ssm = np.asarray(inputs["W_out_ssm"], np.float32)
    W_out = np.asarray(inputs["W_out"], np.float32)
    b_out = np.asarray(inputs["b_out"], np.float32)

    A = -np.exp(A_log)
    W_c = (W_out.astype(np.float64) @ W_out_ssm.astype(np.float64)).astype(np.float32)

    in_maps = []
    for k in range(NC):
        b, r = divmod(k, TPG)
        dsl = slice(r * DIL, (r + 1) * DIL)
        esl = slice(r * EL, (r + 1) * EL)
        in_maps.append({
            "x_t": np.ascontiguousarray(x[b].T.astype(BFnp)),
            "w_in_x": np.ascontiguousarray(W_in[dsl, :].T.astype(BFnp)),
            "w_in_z": np.ascontiguousarray(
                W_in[DI + r * DIL: DI + (r + 1) * DIL, :].T.astype(BFnp)),
            "conv_w": np.ascontiguousarray(conv_w[dsl, 0, :]),
            "conv_b": np.ascontiguousarray(conv_b[dsl][:, None]),
            "ncb_in": np.ascontiguousarray(-conv_b[dsl][:, None]),
            "w_xp": np.ascontiguousarray(W_xp[:, dsl].T.astype(BFnp)),
            "w_dt": np.ascontiguousarray(W_dt[dsl, :].T.astype(BFnp)),
            "b_dt": np.ascontiguousarray(b_dt[dsl][:, None]),
            "a_mat": np.ascontiguousarray(A[dsl, :]),
            "d_vec": np.ascontiguousarray(D[dsl][:, None]),
            "w_c": np.ascontiguousarray(W_c[:, dsl].T.astype(BFnp)),
            "b_o": np.ascontiguousarray(b_out[esl][:, None]),
        })
    return in_maps


def _assemble(results):
    full = np.zeros((B, L, DM), np.float32)
    for k in range(NC):
        b, r = divmod(k, TPG)
        full[b, :, r * EL:(r + 1) * EL] = results[k]["out"].T
    return full


def kernel(**inputs):
    nc = build()
    in_maps = _prep_inputs(inputs)
    res = run_bass_kernel_spmd(nc, in_maps, core_ids=list(range(NC)))
    return _assemble(res.results)


def kernel_sim(**inputs):
    """Run through MultiCoreSim instead of HW (for debugging)."""
    from concourse.bass_interp import MultiCoreSim
    nc = build()
    in_maps = _prep_inputs(inputs)
    sim = MultiCoreSim(nc, num_cores=NC)
    for k in range(NC):
        for name, arr in in_maps[k].items():
            sim.cores[k].tensor(name)[:] = arr
    sim.simulate(check_with_hw=False)
    results = [{"out": sim.cores[k].tensor("out").copy()} for k in range(NC)]
    return _assemble(results)
